# revision 25
# baseline (speedup 1.0000x reference)
"""Trainium2 Bass kernel for the pairwise-KL contrastive loss (nn_KL_Loss).

Reference math (N=512, D=128, 2N=1024):
    mu  = concat(p1_loc, p2_loc)     [2N, D]
    var = concat(p1_scale, p2_scale) [2N, D]
    kld[i,j] = 0.5 * sum_d( lv[j]-lv[i]-1 + ((mu[i]-mu[j])^2 + var[i])/var[j] )
    sim = where(diag, -9e6, kld) * T          (T = 0.01)
    loss = mean_i( sim[i, (i+N)%2N] - logsumexp_j sim[i,:] )

Kernel decomposition (one 128-row block per core):
    2*kld[i,j] = R[i,j] - L[i] - D,  where
    R[i,j] = sum_d A[i,d]*iv[j,d] - 2*sum_d mu[i,d]*(mu*iv)[j,d]
             + sum_d (mu^2*iv)[j,d] + sum_d lv[j,d]
    (A = mu^2 + var, iv = 1/var, lv = log var, L[i] = sum_d lv[i,d])
    -> 4 TensorE matmuls (K = D = 128) accumulated in PSUM per column group.

    The per-row shift -c*(L[i]+D) cancels in sim_pos - logsumexp, so with
    c = 0.5*T:   loss_i = c*R[i,pos] - log( sum_j exp(c*R[i,j]) - exp(c*(L[i]+D)) )
    The subtracted term removes the diagonal (self) entry exactly
    (R[i,i] = L[i]+D).  sim values are O(1) (max ~2.7) so fp32 sum-of-exps
    is stable without max-subtraction.

Layout strategy (all data prep on HOST, which the contract allows --
sharding/gather happen inside kernel()):
  * Inputs are pre-TRANSPOSED on host to [D, 2N] = [128, 1024] so the
    matmul operands (contraction over d on the partition axis) stream
    straight from DRAM -> SBUF with NO on-chip transposes at all.
  * Per core c the columns are rotated by -128c and permuted to
    [own(0:128) | pos-block | rest], so every core runs the identical
    program: self-pairs are the diagonal of columns 0:128 and positive
    pairs the diagonal of columns 128:256 (both in PSUM group 1).
  * Output is a [1, 128] row (one partition, one DMA descriptor) --
    a [128, 1] column would be 128 4-byte descriptors whose completion
    semaphore costs ~6 us extra at kernel end.

Perf notes (from ntff profiles of earlier revisions):
  * Input is HBM-bound, so each tensor loads as two 512-col chunks on
    parallel HWDGE queues (var on sync, mu on scalar).  Finer chunking
    was measured SLOWER twice: each extra chunk adds ~0.7 us of issue
    serialization on its queue, delaying the later chunks more than the
    earlier partial arrival helps.  The gpsimd SWDGE queue delivers its
    completion semaphore ~2.5 us later than HWDGE; avoided.
  * mu is bf16 (products only; loss err ~1e-6) and all derived j-side
    tensors are bf16: DVE muls run at the 2x 16-bit rate and bf16
    matmul moving operands are 1 cyc/col at any width (f32r needs
    >=256 free).  var stays fp32 for the DVE fast-reciprocal bit trick.
  * The diagonal-removal adjust is folded into the final Ln as its
    per-partition bias, cutting one DVE op + a hop from the tail.
  * GPSIMD tensor ops cost ~2 us each ([128,128] incl. lib load), so
    everything elementwise lives on DVE/ACT; GPSIMD only memsets.
  * Remaining time is framework-fixed: ~1.5 us Tile/Bass entry barrier,
    ~1.4 us DMA completion-semaphore observation latency, ~1.9 us
    output issue+join, ~7.5 us NEFF teardown (a do-nothing kernel
    measures ~12.8 us end-to-end on this stack).
"""

import sys
import types

for _p in ("/opt/trn_rl_repo", "/opt/trn_rl_repo/concourse"):
    if _p not in sys.path:
        sys.path.insert(0, _p)

import numpy as np

import bass_rust as _bass_rust
import concourse.bacc as bacc
import concourse.bass as bass  # noqa: F401  (AP helpers)
import concourse.tile as tile
from concourse import mybir
from concourse.bass_utils import run_bass_kernel_spmd
from concourse.hw_specs import get_activation_tables

F32 = mybir.dt.float32
F32R = mybir.dt.float32r
BF16 = mybir.dt.bfloat16
AF = mybir.ActivationFunctionType
ALU = mybir.AluOpType

N2 = 1024  # 2N rows
D = 128
TEMP = 0.01
C = 0.5 * TEMP  # 0.005
N_CORES = 8
N_DUMMY = 8  # PE warm-up matmuls (DVFS ramp) during the input DMA window

_CACHED_NC = None


def _patched_act_table_loads(self):
    """insert_act_table_loads steered so Exp and Ln resolve to the one set
    that has both (`natural_log_exp_and_others`) -> a single ACT_TABLE_LOAD
    instead of thrashing between `exp_and_others` and `natural_log` (~1.3us
    per reload).  The list ORDER must stay untouched (act_func_set_id is the
    index into act_info.json), so instead of reordering we strip Exp/Ln from
    every other set's function list."""
    has_activation = any(
        isinstance(i, mybir.InstActivation)
        for b in self.main_func.blocks
        for i in b.instructions
    )
    if not has_activation:
        return
    keep = "natural_log_exp_and_others"
    tables = [
        (name,
         funcs if name == keep
         else {f for f in funcs if f not in (AF.Exp, AF.Ln)})
        for name, funcs in get_activation_tables(self.m.arch).items()
    ]
    _bass_rust.insert_act_table_loads(self, tables)


def _recip_approx_fast_f32r(nc, out, in_):
    """reciprocal_approx_fast with a float32r-typed output tile.  The wrapper
    in bass asserts fp32 in AND out, but only the *input* needs the fp32 bit
    layout (BITWISE_NOT exponent-flip seed); the output write is a normal DVE
    store which rounds to the out AP's dtype."""
    from concourse.dve_ops import RECIP_APPROX_FAST_CONSTS, RECIPROCAL_APPROX_FAST

    c = RECIP_APPROX_FAST_CONSTS
    return nc.vector._custom_dve(
        RECIPROCAL_APPROX_FAST, out=out, in0=in_,
        s0=c["s0"], s1=c["s1"], imm2=c["imm2"])


def build_nc(loop_n=None):
    from contextlib import nullcontext

    nc = bacc.Bacc(None, target_bir_lowering=False, debug=False)
    nc.insert_act_table_loads = types.MethodType(_patched_act_table_loads, nc)

    # Host supplies transposed + per-core-permuted inputs: [D, 2N].
    mu_d = nc.dram_tensor("muT", [D, N2], BF16, kind="ExternalInput")
    var_d = nc.dram_tensor("varT", [D, N2], F32, kind="ExternalInput")
    loss_d = nc.dram_tensor("loss", [1, 128], F32, kind="ExternalOutput")

    with tile.TileContext(nc) as tc:
        with (
            tc.tile_pool(name="consts", bufs=1) as consts,
            tc.tile_pool(name="nat", bufs=1) as nat,
            tc.tile_pool(name="big", bufs=1) as big,
            tc.tile_pool(name="small", bufs=1) as small,
            tc.tile_pool(name="psum", bufs=1, space="PSUM") as psum,
        ):
            loop_cm = tc.For_i(0, loop_n, 1) if loop_n else nullcontext()
            with loop_cm:
                body(nc, tc, consts, nat, big, small, psum,
                     mu_d, var_d, loss_d)

    nc.compile()
    return nc


def body(nc, tc, consts, nat, big, small, psum, mu_d, var_d, loss_d):
    # ---- constants ----
    ones_f32 = consts.tile([128, 128], F32)
    nc.gpsimd.memset(ones_f32, 1.0)

    # ---- input DMA ----
    # Chunks ordered by need: own/stationary cols 0:128 first, then the
    # rest of PSUM group 1 (incl. the positive-pair diagonal), then the
    # two 256-col groups of the second half.  var on sync, mu on scalar
    # (both HWDGE); emitted before the first activation so the ACT table
    # load runs right after the issue instructions.
    vt = nat.tile([128, N2], F32)
    mt = nat.tile([128, N2], BF16)
    nc.sync.dma_start(out=vt[:, 0:512], in_=var_d[:, 0:512])
    nc.sync.dma_start(out=vt[:, 512:1024], in_=var_d[:, 512:1024])
    nc.scalar.dma_start(out=mt[:, 0:512], in_=mu_d[:, 0:512])
    nc.scalar.dma_start(out=mt[:, 512:1024], in_=mu_d[:, 512:1024])

    ones128 = consts.tile([128, 128], F32R)
    nc.vector.tensor_copy(ones128, ones_f32)
    ones_col = consts.tile([128, 1], F32R)
    nc.vector.tensor_copy(ones_col, ones_f32[:, 0:1])
    ones_col_bf = consts.tile([128, 1], BF16)
    nc.vector.tensor_copy(ones_col_bf, ones_f32[:, 0:1])
    ones128_bf = consts.tile([128, 128], BF16)
    nc.vector.tensor_copy(ones128_bf, ones_f32)
    ident = consts.tile([128, 128], F32)
    # iota[p, x] = p - x ; == 0 on the diagonal
    nc.gpsimd.affine_select(
        out=ident,
        in_=ones_f32,
        pattern=[[-1, 128]],
        base=0,
        channel_multiplier=1,
        compare_op=ALU.is_equal,
        fill=0.0,
    )
    cd_bias = consts.tile([128, 1], F32)
    nc.gpsimd.memset(cd_bias, float(C * D))
    dummy_mv = consts.tile([128, 512], F32)
    nc.gpsimd.memset(dummy_mv, 1.0)
    # ACT warm-up: trigger the (single) exp+ln table load at t~0 so it
    # overlaps the input DMA instead of stalling the first real Ln.
    warm = consts.tile([128, 1], F32)
    nc.scalar.activation(warm, ones_col, AF.Ln)

    # ---- PSUM ----
    p_R1 = psum.tile([128, 512], F32)
    p_R2 = psum.tile([128, 512], F32)
    p_L = psum.tile([128, 1], F32)
    p_lossT = psum.tile([1, 128], F32)
    p_dummy = psum.tile([128, 512], F32)

    # ---- PE warm-up: ramp the tensor-engine clock during the DMA wait ----
    for _ in range(N_DUMMY):
        nc.tensor.matmul(p_dummy, ones128, dummy_mv.bitcast(F32R),
                         start=True, stop=True)

    # ---- derived per-column tensors ----
    lv = big.tile([128, N2], BF16)
    iv = big.tile([128, N2], BF16)
    muiv = big.tile([128, N2], BF16)
    h1 = big.tile([128, N2], BF16)

    nc.scalar.activation(lv[:, 0:512], vt[:, 0:512], AF.Ln)
    nc.scalar.activation(lv[:, 512:1024], vt[:, 512:1024], AF.Ln)

    # own-block stationaries (cols 0:128 = own rows, [d, i] layout) --
    # first on the DVE queue: their input chunk lands first.
    mu2_own = small.tile([128, 128], BF16)  # -2 * mu own block
    nc.vector.tensor_scalar_mul(mu2_own, mt[:, 0:128], -2.0)
    sq_own = small.tile([128, 128], F32)
    nc.vector.scalar_tensor_tensor(
        out=sq_own, in0=mu2_own, scalar=0.25, in1=mu2_own,
        op0=ALU.mult, op1=ALU.mult)
    a_own = small.tile([128, 128], BF16)  # (mu^2 + var) own block
    nc.vector.tensor_add(a_own, vt[:, 0:128], sq_own)

    # DVE chains per column group, in data-arrival order.
    _recip_approx_fast_f32r(nc, out=iv[:, 0:512], in_=vt[:, 0:512])
    nc.vector.tensor_mul(muiv[:, 0:512], mt[:, 0:512], iv[:, 0:512])
    nc.vector.tensor_mul(h1[:, 0:512], muiv[:, 0:512], mt[:, 0:512])
    _recip_approx_fast_f32r(nc, out=iv[:, 512:1024], in_=vt[:, 512:1024])
    nc.vector.tensor_mul(muiv[:, 512:1024], mt[:, 512:1024], iv[:, 512:1024])
    nc.vector.tensor_mul(h1[:, 512:1024], muiv[:, 512:1024], mt[:, 512:1024])

    # ---- main matmuls: R accumulated in PSUM (f32r, 1 cyc/col) ----
    sumexp_c = small.tile([128, 3], F32)

    nc.tensor.matmul(p_R1, ones128_bf, lv[:, 0:512], start=True, stop=False)
    nc.tensor.matmul(p_R1, mu2_own, muiv[:, 0:512], start=False, stop=False)
    nc.tensor.matmul(p_R1, a_own, iv[:, 0:512], start=False, stop=False)
    nc.tensor.matmul(p_R1, ones128_bf, h1[:, 0:512], start=False, stop=True)

    # L_own[i] = sum_d lv[d, i] over the own columns.
    nc.tensor.matmul(p_L, lv[:, 0:128], ones_col_bf, start=True, stop=True)

    exp_scr = big.tile([128, 512], BF16)
    nc.scalar.activation(exp_scr[:, 0:512], p_R1, AF.Exp, scale=C,
                         accum_out=sumexp_c[:, 0:1])
    diag_exp = small.tile([128, 1], F32)
    nc.scalar.activation(diag_exp, p_L, AF.Exp, scale=C, bias=cd_bias)

    nc.tensor.matmul(p_R2, ones128_bf, lv[:, 512:1024], start=True, stop=False)
    nc.tensor.matmul(p_R2, mu2_own, muiv[:, 512:1024], start=False, stop=False)
    nc.tensor.matmul(p_R2, a_own, iv[:, 512:1024], start=False, stop=False)
    nc.tensor.matmul(p_R2, ones128_bf, h1[:, 512:1024], start=False, stop=True)

    # positive-pair extraction: diag of R1[:, 128:256] (pre-exp values).
    pos_scr = small.tile([128, 128], F32)
    pos_raw = small.tile([128, 1], F32)
    nc.vector.tensor_mul(pos_scr, p_R1[:, 128:256], ident)
    nc.vector.reduce_sum(pos_raw, pos_scr, axis=mybir.AxisListType.X)

    exp_scr2 = big.tile([128, 512], BF16)
    nc.scalar.activation(exp_scr2, p_R2, AF.Exp, scale=C,
                         accum_out=sumexp_c[:, 1:2])

    # ln(sum_j exp) with the diagonal removed: pre = group1 - self_exp is
    # computed as soon as the first accumulator lands, then folded into the
    # final Ln as its per-partition bias: ln_s = Ln(group2 + pre).
    pre_adj = small.tile([128, 1], F32)
    nc.vector.tensor_sub(pre_adj, sumexp_c[:, 0:1], diag_exp)
    log_s = small.tile([128, 1], F32)
    nc.scalar.activation(log_s, sumexp_c[:, 1:2], AF.Ln, bias=pre_adj)
    loss_sb = small.tile([128, 1], F32)
    nc.vector.scalar_tensor_tensor(
        out=loss_sb,
        in0=pos_raw,
        scalar=float(C),
        in1=log_s,
        op0=ALU.mult,
        op1=ALU.subtract,
    )

    # Transpose to one partition so the output DMA is a single descriptor.
    nc.tensor.transpose(p_lossT, loss_sb, ident)
    loss_row = small.tile([1, 128], F32)
    nc.vector.tensor_copy(loss_row, p_lossT)
    nc.sync.dma_start(out=loss_d[:], in_=loss_row)


# Per-core column permutation: [own 0:128 | pos block | remaining].
_P = np.concatenate([np.arange(0, 128), np.arange(512, 1024),
                     np.arange(128, 512)]).astype(np.int64)


def run_spmd(p1_loc, p2_loc, p1_scale, p2_scale, **spmd_kwargs):
    """Shard, run on 8 cores, gather.  Returns (loss_scalar, BassKernelResults)."""
    global _CACHED_NC
    import ml_dtypes
    mu_t = np.concatenate([p1_loc, p2_loc], axis=0).astype(np.float32).T
    var_t = np.concatenate([p1_scale, p2_scale], axis=0).astype(np.float32).T
    mu_t = np.ascontiguousarray(mu_t).astype(ml_dtypes.bfloat16)  # [D, 2N]
    var_t = np.ascontiguousarray(var_t)
    if _CACHED_NC is None:
        _CACHED_NC = build_nc()
    nc = _CACHED_NC
    in_maps = []
    for c in range(N_CORES):
        cols = (_P + 128 * c) % N2
        in_maps.append({
            "muT": np.ascontiguousarray(mu_t[:, cols]),
            "varT": np.ascontiguousarray(var_t[:, cols]),
        })
    res = run_bass_kernel_spmd(nc, in_maps, core_ids=list(range(N_CORES)),
                               **spmd_kwargs)
    rows = np.concatenate([r["loss"].reshape(-1) for r in res.results])
    return np.array(rows.mean(), dtype=np.float32), res


def kernel(p1_loc, p2_loc, p1_scale, p2_scale):
    loss, _ = run_spmd(p1_loc, p2_loc, p1_scale, p2_scale)
    return loss


if __name__ == "__main__":
    import reference

    inputs = reference.setup_inputs()
    expected = np.asarray(reference.reference(**inputs))
    actual = kernel(**{k: np.asarray(v) for k, v in inputs.items()})
    rel = abs(float(actual) - float(expected)) / max(abs(float(expected)), 1e-30)
    print("expected:", expected, "actual:", actual, "rel err:", rel)
